# revision 14
# baseline (speedup 1.0000x reference)
"""Multi-head attention (dense transformer block) on 8 Trainium2 NeuronCores.

Sharding: pure data-parallel over (batch=4) x (query half=2) -> 8 shards.
Each core computes, for its batch element b and query-token half:
  Qt = (Wq @ x_b.T)[:, half]  (transposed layout, dmodel x 1024)
  Kt = Wk @ x_b.T             (dmodel x 2048)
  V  = x_b @ Wv.T             (natural layout, interleaved with a ones
                               column per head for the softmax denominator)
  per head: St = K.Q (scores transposed, k on partitions), Pt = exp(St/8),
            O_unnorm = V.T@Pt and Z = ones.T@Pt via one fused matmul pair,
            Ot = O_unnorm * (1/Z broadcast)
  out = Ot.T @ Wo.T + bo      (natural layout, written to DRAM)

K/V are computed redundantly by the two cores sharing a batch element; no
collectives are needed and every core writes a disjoint output slice.
"""

import numpy as np
import ml_dtypes

import concourse.bass as bass
import concourse.tile as tile
import concourse.mybir as mybir
from concourse.bass_utils import run_bass_kernel_spmd

F32 = mybir.dt.float32
F32R = mybir.dt.float32r
BF16 = mybir.dt.bfloat16
EXP = mybir.ActivationFunctionType.Exp

D = 1024          # d_model
S = 2048          # sequence length
NH = 16           # heads
DH = 64           # head dim
QL = 1024         # query rows per core
NCORES = 8


def split_multi_waits(nc):
    """The walrus build in this container accepts at most one sync-wait per
    instruction; move extra waits onto same-engine nops inserted before the
    offending instruction."""
    k = 0
    for f in nc.m.functions:
        for bb in f.blocks:
            out, changed = [], False
            for inst in bb.instructions:
                si = inst.sync_info
                waits = list(si.on_wait) if si and si.on_wait else []
                if len(waits) > 1:
                    changed = True
                    for w in waits[:-1]:
                        nop = mybir.InstNoOp(name=f"wsplit-{k}", ins=[], outs=[])
                        k += 1
                        nop.engine = inst.engine
                        nop.sync_info = mybir.SyncInfo(on_wait=[w], on_update=[])
                        nc.register_instruction(nop, overwrite=True)
                        out.append(nop)
                    si.on_wait = waits[-1:]
                out.append(inst)
            if changed:
                bb.instructions = out


def r32(ap):
    return ap.bitcast(F32R)


def build_program(repeat=1):
    nc = bass.Bass()
    xqT = nc.declare_dram_parameter("xqT", [D, QL], F32R, isOutput=False)
    xT = nc.declare_dram_parameter("xT", [D, S], F32R, isOutput=False)
    wqT = nc.declare_dram_parameter("wqT", [D, D], F32R, isOutput=False)
    wkT = nc.declare_dram_parameter("wkT", [D, D], F32R, isOutput=False)
    wvT = nc.declare_dram_parameter("wvT", [D, D], F32R, isOutput=False)
    woT = nc.declare_dram_parameter("woT", [D, D], BF16, isOutput=False)
    bq2 = nc.declare_dram_parameter("bq2", [128, 8], F32, isOutput=False)
    bk2 = nc.declare_dram_parameter("bk2", [128, 8], F32, isOutput=False)
    bvb = nc.declare_dram_parameter("bvb", [128, D], F32, isOutput=False)
    bob = nc.declare_dram_parameter("bob", [128, D], F32, isOutput=False)
    mask2 = nc.declare_dram_parameter("mask2", [2, 128], F32R, isOutput=False)
    out = nc.declare_dram_parameter("out", [QL, D], F32, isOutput=True)

    with tile.TileContext(nc) as tc:
        if repeat > 1:
            ctx = tc.For_i(0, repeat, 1)
        else:
            import contextlib
            ctx = contextlib.nullcontext()
        with ctx, tc.tile_pool(name="persist", bufs=1) as pp:
            qt = [pp.tile([128, QL], BF16, name=f"qt{p}", tag=f"qt{p}") for p in range(8)]
            kt = [pp.tile([128, S], BF16, name=f"kt{p}", tag=f"kt{p}") for p in range(8)]
            vg = [pp.tile([128, NH * (DH + 1)], BF16, name=f"vg{t}", tag=f"vg{t}")
                  for t in range(16)]
            ot = [pp.tile([128, QL], BF16, name=f"ot{p}", tag=f"ot{p}") for p in range(8)]
            bq_sb = pp.tile([128, 8], F32, name="bq_sb", tag="bq_sb")
            bk_sb = pp.tile([128, 8], F32, name="bk_sb", tag="bk_sb")
            bvb_sb = pp.tile([128, D], F32, name="bvb_sb", tag="bvb_sb")
            bob_sb = pp.tile([128, D], F32, name="bob_sb", tag="bob_sb")
            mask_sb = pp.tile([128, 128], F32R, name="mask_sb", tag="mask_sb")
            nc.sync.dma_start(mask_sb[0:2, :], mask2[:])
            nc.sync.dma_start(bq_sb[:], bq2[:])
            nc.sync.dma_start(bk_sb[:], bk2[:])
            nc.sync.dma_start(bvb_sb[:], bvb[:])
            nc.sync.dma_start(bob_sb[:], bob[:])

            # ---- Phase A1: Q projection (transposed): Qt[dout, t] over our
            # 1024 query tokens.
            with tc.tile_pool(name="wq", bufs=1) as wp, \
                 tc.tile_pool(name="xq", bufs=2) as xp, \
                 tc.tile_pool(name="psA", bufs=4, space="PSUM") as psp:
                wq_sb = [wp.tile([128, D], F32R, name=f"wq{d}", tag=f"wq{d}")
                         for d in range(8)]
                for d in range(8):
                    nc.sync.dma_start(wq_sb[d][:], wqT[128 * d:128 * (d + 1), :])
                for qb in range(2):
                    xq_sb = [xp.tile([128, 512], F32R, name=f"xq{d}", tag=f"xq{d}")
                             for d in range(8)]
                    for d in range(8):
                        nc.sync.dma_start(
                            xq_sb[d][:], xqT[128 * d:128 * (d + 1), 512 * qb:512 * (qb + 1)])
                    for p in range(8):
                        ps = psp.tile([128, 512], F32, name="psq", tag="ps", bufs=4)
                        for d in range(8):
                            nc.tensor.matmul(
                                ps[:], wq_sb[d][:, 128 * p:128 * (p + 1)],
                                xq_sb[d][:], start=(d == 0), stop=(d == 7))
                        nc.scalar.add(qt[p][:, 512 * qb:512 * (qb + 1)], ps[:],
                                      bq_sb[:, p:p + 1])

            # ---- Phase A2: K projection (transposed): Kt[dout, t] over all
            # 2048 tokens, stored bf16.
            with tc.tile_pool(name="wk", bufs=1) as wp, \
                 tc.tile_pool(name="xk", bufs=2) as xp, \
                 tc.tile_pool(name="psA2", bufs=4, space="PSUM") as psp:
                wk_sb = [wp.tile([128, D], F32R, name=f"wk{d}", tag=f"wk{d}")
                         for d in range(8)]
                for d in range(8):
                    nc.sync.dma_start(wk_sb[d][:], wkT[128 * d:128 * (d + 1), :])
                for tb in range(4):
                    xk_sb = [xp.tile([128, 512], F32R, name=f"xk{d}", tag=f"xk{d}")
                             for d in range(8)]
                    for d in range(8):
                        nc.sync.dma_start(
                            xk_sb[d][:], xT[128 * d:128 * (d + 1), 512 * tb:512 * (tb + 1)])
                    for p in range(8):
                        ps = psp.tile([128, 512], F32, name="psk", tag="ps", bufs=4)
                        for d in range(8):
                            nc.tensor.matmul(
                                ps[:], wk_sb[d][:, 128 * p:128 * (p + 1)],
                                xk_sb[d][:], start=(d == 0), stop=(d == 7))
                        nc.scalar.add(kt[p][:, 512 * tb:512 * (tb + 1)], ps[:],
                                      bk_sb[:, p:p + 1])

            # ---- Phase B: V projection (natural layout, per-head 65-column
            # interleave with a trailing ones column for the softmax sum).
            with tc.tile_pool(name="wv", bufs=1) as wp, \
                 tc.tile_pool(name="xv", bufs=2) as xp, \
                 tc.tile_pool(name="psB", bufs=4, space="PSUM") as psp:
                wv_sb = [wp.tile([128, D], F32R, name=f"wv{d}", tag=f"wv{d}")
                         for d in range(8)]
                for d in range(8):
                    nc.sync.dma_start(wv_sb[d][:], wvT[128 * d:128 * (d + 1), :])
                for tb in range(4):
                    xv_sb = [xp.tile([128, 512], F32R, name=f"xv{d}", tag=f"xv{d}")
                             for d in range(8)]
                    for d in range(8):
                        nc.sync.dma_start(
                            xv_sb[d][:], xT[128 * d:128 * (d + 1), 512 * tb:512 * (tb + 1)])
                    for sub in range(4):
                        ti = 4 * tb + sub
                        for hf in range(2):
                            ps = psp.tile([128, 512], F32, name="psv", tag="ps", bufs=4)
                            for d in range(8):
                                nc.tensor.matmul(
                                    ps[:], xv_sb[d][:, 128 * sub:128 * (sub + 1)],
                                    wv_sb[d][:, 512 * hf:512 * (hf + 1)],
                                    start=(d == 0), stop=(d == 7))
                            dst = vg[ti][:, 520 * hf:520 * (hf + 1)].rearrange(
                                "p (h w) -> p h w", w=65)[:, :, 0:64]
                            nc.vector.tensor_add(
                                dst,
                                ps[:].rearrange("p (h w) -> p h w", w=64),
                                bvb_sb[:, 512 * hf:512 * (hf + 1)].rearrange(
                                    "p (h w) -> p h w", w=64))
                        nc.vector.memset(
                            vg[ti][:].rearrange("p (h w) -> p h w", w=65)[:, :, 64:65],
                            1.0)

            # ---- Phase C: attention, head pair p = heads (2p, 2p+1).
            with tc.tile_pool(name="pt", bufs=4) as ptp, \
                 tc.tile_pool(name="rz", bufs=2) as rzp, \
                 tc.tile_pool(name="psSt", bufs=4, space="PSUM") as stp, \
                 tc.tile_pool(name="psO", bufs=2, space="PSUM") as pop, \
                 tc.tile_pool(name="psZ", bufs=2, space="PSUM") as pzp:
                for p in range(8):
                    c0 = 130 * p          # head 2p columns in vg
                    c1 = 130 * p + 65     # head 2p+1 columns in vg
                    for qb in range(2):
                        qs = slice(512 * qb, 512 * (qb + 1))
                        po = pop.tile([128, 512], F32, name="po", tag="po", bufs=2)
                        pz = pzp.tile([128, 512], F32, name="pz", tag="pz", bufs=2)
                        for k in range(16):
                            ks = slice(128 * k, 128 * (k + 1))
                            st0 = stp.tile([128, 512], F32, name="st0", tag="st", bufs=4)
                            st1 = stp.tile([128, 512], F32, name="st1", tag="st", bufs=4)
                            nc.tensor.matmul(st0[:], kt[p][0:64, ks], qt[p][0:64, qs],
                                             start=True, stop=True)
                            nc.tensor.matmul(st1[:], kt[p][64:128, ks], qt[p][64:128, qs],
                                             start=True, stop=True)
                            pt0 = ptp.tile([128, 512], BF16, name="pt0", tag="pt", bufs=4)
                            pt1 = ptp.tile([128, 512], BF16, name="pt1", tag="pt", bufs=4)
                            nc.scalar.activation(pt0[:], st0[:], EXP, scale=0.125)
                            nc.scalar.activation(pt1[:], st1[:], EXP, scale=0.125)
                            first, last = (k == 0), (k == 15)
                            nc.tensor.matmul(po[0:64, :], vg[k][:, c0:c0 + 64],
                                             pt0[:], start=first, stop=last,
                                             skip_group_check=True)
                            nc.tensor.matmul(po[64:128, :], vg[k][:, c1:c1 + 64],
                                             pt1[:], start=first, stop=last,
                                             skip_group_check=True)
                            nc.tensor.matmul(pz[0:1, :], vg[k][:, c0 + 64:c0 + 65],
                                             pt0[:], start=first, stop=last,
                                             skip_group_check=True)
                            nc.tensor.matmul(pz[32:33, :], vg[k][:, c1 + 64:c1 + 65],
                                             pt1[:], start=first, stop=last,
                                             skip_group_check=True)
                        rz = rzp.tile([128, 512], F32R, name="rz", tag="rz", bufs=2)
                        with nc.allow_low_precision(reason="1/Z fed to f32r matmul"):
                            nc.vector.reciprocal(rz[0:1, :], pz[0:1, :])
                            nc.vector.reciprocal(rz[32:33, :], pz[32:33, :])
                        # engines can only write 32-aligned partition bases;
                        # DMA-hop row 32 down to row 1 for the K=2 matmul.
                        nc.sync.dma_start(rz[1:2, :], rz[32:33, :])
                        # broadcast 1/Z along the head dim via one K=2 PE outer
                        # product: pb[d, q] = mask[0,d]*rz0[q] + mask[1,d]*rz1[q]
                        pb = pzp.tile([128, 512], F32, name="pb", tag="pz", bufs=2)
                        nc.tensor.matmul(pb[:], mask_sb[0:2, :], rz[0:2, :],
                                         start=True, stop=True)
                        rb = rzp.tile([128, 512], F32, name="rb", tag="rb", bufs=2)
                        nc.vector.tensor_copy(rb[:], pb[:])
                        nc.vector.tensor_mul(ot[p][:, qs], po[:], rb[:])

            # ---- Phase D: output projection + bias, natural layout.
            with tc.tile_pool(name="wo", bufs=1) as wp, \
                 tc.tile_pool(name="osb", bufs=3) as op_, \
                 tc.tile_pool(name="psD", bufs=4, space="PSUM") as psp:
                wo_sb = [wp.tile([128, D], BF16, name=f"wo{d}", tag=f"wo{d}")
                         for d in range(8)]
                for d in range(8):
                    nc.sync.dma_start(wo_sb[d][:], woT[128 * d:128 * (d + 1), :])
                for t8 in range(8):
                    for hf in range(2):
                        ps = psp.tile([128, 512], F32, name="pso", tag="ps", bufs=4)
                        for p in range(8):
                            nc.tensor.matmul(
                                ps[:], ot[p][:, 128 * t8:128 * (t8 + 1)],
                                wo_sb[p][:, 512 * hf:512 * (hf + 1)],
                                start=(p == 0), stop=(p == 7))
                        osb = op_.tile([128, 512], F32, name="osb", tag="osb", bufs=3)
                        nc.vector.tensor_add(osb[:], ps[:], bob_sb[:, 512 * hf:512 * (hf + 1)])
                        nc.sync.dma_start(
                            out[128 * t8:128 * (t8 + 1), 512 * hf:512 * (hf + 1)], osb[:])

    split_multi_waits(nc)
    return nc


_CACHED_NC = None


def get_program():
    global _CACHED_NC
    if _CACHED_NC is None:
        _CACHED_NC = build_program()
    return _CACHED_NC


def make_in_maps(x, Wq, bq, Wk, bk, Wv, bv, Wo, bo):
    x = np.asarray(x, np.float32)
    shared = {
        "wqT": np.ascontiguousarray(np.asarray(Wq, np.float32).T),
        "wkT": np.ascontiguousarray(np.asarray(Wk, np.float32).T),
        "wvT": np.ascontiguousarray(np.asarray(Wv, np.float32).T),
        "woT": np.ascontiguousarray(np.asarray(Wo, np.float32).T).astype(ml_dtypes.bfloat16),
        "bq2": np.ascontiguousarray(np.asarray(bq, np.float32).reshape(8, 128).T),
        "bk2": np.ascontiguousarray(np.asarray(bk, np.float32).reshape(8, 128).T),
        "bvb": np.ascontiguousarray(np.tile(np.asarray(bv, np.float32), (128, 1))),
        "bob": np.ascontiguousarray(np.tile(np.asarray(bo, np.float32), (128, 1))),
        "mask2": np.ascontiguousarray(
            np.repeat(np.eye(2, dtype=np.float32), 64, axis=1)),
    }
    in_maps = []
    for c in range(NCORES):
        b, half = c // 2, c % 2
        m = dict(shared)
        m["xT"] = np.ascontiguousarray(x[b].T)
        m["xqT"] = np.ascontiguousarray(x[b, half * QL:(half + 1) * QL].T)
        in_maps.append(m)
    return in_maps


def kernel(x, Wq, bq, Wk, bk, Wv, bv, Wo, bo):
    nc = get_program()
    in_maps = make_in_maps(x, Wq, bq, Wk, bk, Wv, bv, Wo, bo)
    res = run_bass_kernel_spmd(nc, in_maps, list(range(NCORES)))
    out = np.empty((4, S, D), np.float32)
    for c in range(NCORES):
        b, half = c // 2, c % 2
        out[b, half * QL:(half + 1) * QL, :] = res.results[c]["out"]
    return out
